# revision 25
# baseline (speedup 1.0000x reference)
"""AdaptiveBoundaryRankingLoss on 8 TRN2 NeuronCores — fp8 DoubleRow rank-10.

Math: loss = sum_{i<j} relu(boundary(|dt|) - (p_i-p_j)*sign(dt)) / K,
  dt = t_i - t_j, boundary(a) = BETA*a/(1+GAMMA*a), K = B(B-1)/2.

Host sorts (pred,target) by target ascending (the loss is a sum over
unordered pairs, so relabeling is free). After sorting, for i>j
(strict lower triangle) sign(t_i - t_j) = +1, so with
m(a) = a/(1+GAMMA*a), a = t_i - t_j >= 0, pc = p/BETA:
  loss = BETA/K * sum_{i>j} relu(m(a) + pc_j - pr_i).

m(a) is approximated per row by a minimax quadratic on a in [0, L_row]:
  m(a) ~= beta_r - (s_r*(a - a0_r))^2.
Expanding the square, the pre-relu value is a rank-4 bilinear form
  z_ij = 1*pc_j + k1_i*t_j^2 + k2_i*t_j + k3_i
(k1 = -s^2, k2 = 2sb, k3 = ubt - b^2, b = s(t_i - a0), ubt = beta_r - pr).
TensorE computes z into PSUM f32. To run the PE at fp8 DoubleRow speed
(0.5 cycles/row — the per-buffer TE-fill must hide inside the consumer
window) each factor is split fp8-main + fp8-residual, and the four
logical products expand to 10 first-order products = one K_phys=5,
2-ktile DoubleRow matmul per 512 columns. Second-order (res*res) terms
are dropped (~0.4% of a term, bf16-like accuracy).

Triangular masking: the basis pc rows are baked PER TILE with
pc = -240 (fp8 max is 240) for columns >= 128*(row_block+1), so padded
columns give z < 0 and relu -> 0 exactly. The remaining over-count (the
j >= i half of each block's own 128x128 diagonal square) is computed on
the host from the same fp8 values and subtracted.

PSUM consumption (relu + row-sum in ONE fused op per 2048-col chunk):
  ScalarE: activation(Relu, accum_out)     — 9 of 18 chunks (evens)
  VectorE: tensor_scalar(max 0, accum_out) — 9 of 18 chunks (odds)
Two [128,2048] PSUM buffers (4 banks each) double-buffer TensorE against
the consumers. Per-chunk row-sums land in acc[128,18] f32, DMA'd out;
the host does the final reduce and subtracts the triangle correction.

Work split: 64 row-blocks of 128 rows; core c takes blocks {8k+c},
tile k spans columns [0,(k+1)*1024) -> identical graph on all cores
(SPMD); per-core differences live in input data (basis + coeffs).
The kernel executes the NEFF twice and returns the second (warm) run.
"""

import contextlib

import numpy as np
import ml_dtypes

import concourse.bass as bass
from concourse import mybir
from concourse.bass_utils import run_bass_kernel_spmd

B = 8192
BETA = 0.3
GAMMA = 0.1
NCORES = 8
NT = 8            # tiles per core (one 128-row block each)
P = 128
TOT = 36864       # sum of tile widths (k+1)*1024
CHUNK = 2048      # consumer chunk width (4 PSUM banks)
NCH = TOT // CHUNK  # 18
MMW = 512         # matmul moving max (output columns per matmul)
KR = 5            # K_phys of the DoubleRow matmul (10 effective rows)
MASK = -240.0     # fp8e4 most-negative finite

# tile column offsets in the concatenated basis
OFFS = [0]
for _k in range(NT):
    OFFS.append(OFFS[-1] + (_k + 1) * 1024)  # [0,1024,3072,...,36864]

# every chunk is consumed by BOTH engines in parallel: ScalarE takes
# cols [0, SEW), VectorE [SEW, CHUNK) — per-chunk latencies ~equal
# (SE has the bigger fixed overhead: accum-readout is a separate 285ns op)
SEW = 960

# basis is stored per tile (ktile stride = tile width must fit a 16-bit
# ISA step field); DMA'd tile by tile in order

_bf16 = ml_dtypes.bfloat16
_f8 = ml_dtypes.float8_e4m3

_NC_CACHE = None


def _tile_of(col):
    for k in range(NT):
        if col < OFFS[k + 1]:
            return k
    raise ValueError(col)


def build_nc():
    nc = bass.Bass(target_bir_lowering=False, debug=False)
    f32 = mybir.dt.float32
    bf16 = mybir.dt.bfloat16
    f8 = mybir.dt.float8e4
    A = mybir.AluOpType

    # head = coef (NT*P) ++ basis tile0 (1024) ++ basis tile1 (2048):
    # one DMA covers everything the first ~3.4us of matmuls need
    HEADW = NT * P + 1024 + 2048
    basis_d = nc.declare_dram_parameter("basis", [KR, 2, TOT], f8, isOutput=False)
    head_d = nc.declare_dram_parameter("head", [KR, 2, HEADW], f8, isOutput=False)
    out_d = nc.declare_dram_parameter("out", [P, 2 * NCH], f32, isOutput=True)

    es = contextlib.ExitStack()
    with es:
        def sb(name, shape, dtype):
            return es.enter_context(nc.sbuf_tensor(name, shape, dtype))

        head = sb("head_s", [KR, 2, HEADW], f8)
        coef = head[:, :, :NT * P]
        basis_t = [
            head[:, :, NT * P:NT * P + 1024],
            head[:, :, NT * P + 1024:],
        ] + [
            sb(f"basis_s{k}", [KR, 2, (k + 1) * 1024], f8)
            for k in range(2, NT)
        ]
        scr_se = sb("scr_se", [P, SEW], bf16)
        scr_ve = sb("scr_ve", [P, CHUNK - SEW], bf16)
        acc = sb("acc", [P, 2 * NCH], f32)
        pa = es.enter_context(nc.psum_tensor("pa", [P, CHUNK], f32))
        pb = es.enter_context(nc.psum_tensor("pb", [P, CHUNK], f32))
        dma_sem = es.enter_context(nc.semaphore("dma_sem"))
        dma_b = es.enter_context(nc.semaphore("dma_b"))
        te_sem = es.enter_context(nc.semaphore("te_sem"))
        se_sem = es.enter_context(nc.semaphore("se_sem"))
        ve_sem = es.enter_context(nc.semaphore("ve_sem"))
        block = es.enter_context(nc.Block())

        pbufs = [pa, pb]

        @block.sync
        def _(sync):
            # head first (coef + tiles 0-1), then odd remaining tiles;
            # even remaining tiles ride the gpsimd queue in parallel
            sync.dma_start(out=head[:, :, :], in_=head_d[:, :, :]).then_inc(
                dma_sem, 16)
            for k in (3, 5, 7):
                lo, hi = OFFS[k], OFFS[k + 1]
                sync.dma_start(
                    out=basis_t[k][:, :, :], in_=basis_d[:, :, lo:hi]
                ).then_inc(dma_sem, 16)
            sync.wait_ge(se_sem, NCH)
            sync.wait_ge(ve_sem, NCH)
            sync.dma_start(out=out_d[:, :], in_=acc[:, :]).then_inc(dma_sem, 16)

        @block.gpsimd
        def _(gpsimd):
            for k in (2, 4, 6):
                lo, hi = OFFS[k], OFFS[k + 1]
                gpsimd.dma_start(
                    out=basis_t[k][:, :, :], in_=basis_d[:, :, lo:hi]
                ).then_inc(dma_b, 16)

        # DMA sem threshold a tile's matmuls must wait for, per queue
        DMA_Q = {0: (dma_sem, 16), 1: (dma_sem, 16),
                 3: (dma_sem, 32), 5: (dma_sem, 48), 7: (dma_sem, 64),
                 2: (dma_b, 16), 4: (dma_b, 32), 6: (dma_b, 48)}

        @block.tensor
        def _(tensor):
            tensor.wait_ge(dma_sem, 16)  # head (coef + tiles 0-1)
            seen_tile = -1
            for c in range(NCH):
                # buffer reuse: wait until chunk c-2's consumers are done
                if c >= 2:
                    tensor.wait_ge(se_sem, c - 1)
                    tensor.wait_ge(ve_sem, c - 1)
                ps = pbufs[c % 2]
                for s in range(CHUNK // MMW):
                    col = c * CHUNK + s * MMW
                    k = _tile_of(col)
                    if k > seen_tile:
                        seen_tile = k
                        sem, thr = DMA_Q[k]
                        tensor.wait_ge(sem, thr)
                    mm = tensor.matmul(
                        ps[:, s * MMW:(s + 1) * MMW],
                        coef[:, :, k * P:(k + 1) * P],
                        basis_t[k][:, :, col - OFFS[k]:col - OFFS[k] + MMW],
                        start=True,
                        stop=True,
                        perf_mode=mybir.MatmulPerfMode.DoubleRow,
                    )
                    # inc only twice per chunk (SE gate at s=1, VE at s=3):
                    # every then_inc costs an event channel + postamble reset
                    if s in (1, 3):
                        mm.then_inc(te_sem, 1)

        @block.scalar
        def _(scalar):
            # dummy 1-elem Relu: pulls ACT_TABLE_LOAD to t=0
            scalar.activation(
                scr_se[:, 0:1], scr_se[:, 0:1],
                mybir.ActivationFunctionType.Relu,
            )
            for c in range(NCH):
                # SE needs only the first SEW cols = first 2 matmuls
                scalar.wait_ge(te_sem, 2 * c + 1)
                scalar.activation(
                    scr_se[:, :], pbufs[c % 2][:, :SEW],
                    mybir.ActivationFunctionType.Relu,
                    accum_out=acc[:, 2 * c:2 * c + 1],
                ).then_inc(se_sem, 1)

        @block.vector
        def _(vector):
            for c in range(NCH):
                vector.wait_ge(te_sem, 2 * (c + 1))
                vector.tensor_scalar(
                    out=scr_ve[:, :], in0=pbufs[c % 2][:, SEW:],
                    scalar1=0.0, scalar2=None, op0=A.max, op1=A.add,
                    accum_out=acc[:, 2 * c + 1:2 * c + 2],
                ).then_inc(ve_sem, 1)

    return nc


def _get_nc():
    global _NC_CACHE
    if _NC_CACHE is None:
        _NC_CACHE = build_nc()
    return _NC_CACHE


def _quad_fit_rows(L, n=48):
    """Vectorized per-row quadratic fit of m(a)=a/(1+G*a) on [0, L_r]
    via Chebyshev interpolation (degree 2). Returns coeff arrays
    (c0, c1, c2) of p(a) = c0 + c1*a + c2*a^2."""
    L = np.maximum(np.asarray(L, np.float64), 1e-3)
    n_ = n
    xk = np.cos((2 * np.arange(n_) + 1) * np.pi / (2 * n_))
    a = (xk[None, :] + 1.0) * 0.5 * L[:, None]          # [rows, n]
    f = a / (1.0 + GAMMA * a)
    b0 = f @ (np.ones_like(xk) / n_)
    b1 = f @ (xk * 2.0 / n_)
    b2 = f @ ((2 * xk * xk - 1.0) * 2.0 / n_)
    # p(x) = (b0 - b2) + b1*x + 2*b2*x^2,  x = 2a/L - 1
    A0 = b0 - b2
    A1 = b1
    A2 = 2 * b2
    c0 = A0 - A1 + A2
    c1 = (A1 - 2 * A2) * 2.0 / L
    c2 = A2 * 4.0 / (L * L)
    return c0, c1, c2


def _split8(x):
    """fp8 main + fp8 residual decomposition (returns fp8 arrays)."""
    m = np.asarray(x, np.float64).astype(_f8)
    r = (np.asarray(x, np.float64) - m.astype(np.float64)).astype(_f8)
    return m, r


def _make_in_maps(pred, target):
    """Returns (in_maps, corr) where corr is the host-side sum of
    relu(z) over the j>=i part of every block's diagonal 128x128
    square (exactly what the device over-counts)."""
    order = np.argsort(target, kind="stable")
    t = target[order].astype(np.float64)
    p = pred[order].astype(np.float64)
    tmin = t[0]

    # shared basis data: fp8 main + residual of pc, t^2, t
    pc_m, pc_r = _split8(p / BETA)
    t2_m, t2_r = _split8(t * t)
    t_m, t_r = _split8(t)
    ones8 = np.ones(B, dtype=_f8)
    # f64 views for the host correction
    pc_mf, pc_rf = pc_m.astype(np.float64), pc_r.astype(np.float64)
    t2_mf, t2_rf = t2_m.astype(np.float64), t2_r.astype(np.float64)
    t_mf, t_rf = t_m.astype(np.float64), t_r.astype(np.float64)

    in_maps = []
    corr = 0.0
    jj = np.arange(P)
    tri = jj[None, :] >= jj[:, None]     # within-block j >= i (incl diag)
    for c in range(NCORES):
        rows = (8 * np.arange(NT)[None, :] + c) * P + np.arange(P)[:, None]
        tr = t[rows]                      # [128, 8]
        pr = p[rows] / BETA
        c0, c1, c2 = _quad_fit_rows((tr - tmin).ravel())
        c0 = c0.reshape(P, NT)
        c1 = c1.reshape(P, NT)
        c2 = np.minimum(c2.reshape(P, NT), -1e-8)
        s = np.sqrt(-c2)
        a0 = -c1 / (2 * c2)
        beta_r = c0 - c2 * a0 * a0
        b = s * (tr - a0)
        ubt = beta_r - pr
        k1m, k1r = _split8(-(s * s))      # [128, 8] each
        k2m, k2r = _split8(2 * s * b)
        k3m, k3r = _split8(ubt - b * b)

        # coef rows (K_phys=5, 2 ktiles): pairing per module docstring
        coef = np.zeros((KR, 2, NT * P), dtype=_f8)
        for k in range(NT):
            sl = slice(k * P, (k + 1) * P)
            coef[0, 0, sl] = _f8(1.0)
            coef[0, 1, sl] = _f8(1.0)
            coef[1, 0, sl] = k1m[:, k]
            coef[1, 1, sl] = k1m[:, k]
            coef[2, 0, sl] = k1r[:, k]
            coef[2, 1, sl] = k2m[:, k]
            coef[3, 0, sl] = k2m[:, k]
            coef[3, 1, sl] = k2r[:, k]
            coef[4, 0, sl] = k3m[:, k]
            coef[4, 1, sl] = k3r[:, k]

        basis = np.empty((KR, 2, TOT), dtype=_f8)
        for k in range(NT):
            lo, hi = OFFS[k], OFFS[k + 1]
            w = hi - lo
            r = 8 * k + c
            jmax = P * (r + 1)            # valid cols are j < jmax
            pm = pc_m[:w].copy()
            prs = pc_r[:w].copy()
            if jmax < w:
                pm[jmax:] = _f8(MASK)
                prs[jmax:] = _f8(MASK)
            basis[0, 0, lo:hi] = pm
            basis[0, 1, lo:hi] = prs
            basis[1, 0, lo:hi] = t2_m[:w]
            basis[1, 1, lo:hi] = t2_r[:w]
            basis[2, 0, lo:hi] = t2_m[:w]
            basis[2, 1, lo:hi] = t_r[:w]
            basis[3, 0, lo:hi] = t_m[:w]
            basis[3, 1, lo:hi] = t_m[:w]
            basis[4, 0, lo:hi] = ones8[:w]
            basis[4, 1, lo:hi] = ones8[:w]

            # host correction for this block's diagonal square
            j0 = P * r
            js = slice(j0, j0 + P)
            zsq = (
                pc_mf[js][None, :] + pc_rf[js][None, :]
                + k1m[:, k].astype(np.float64)[:, None]
                * (t2_mf[js] + t2_rf[js])[None, :]
                + k1r[:, k].astype(np.float64)[:, None] * t2_mf[js][None, :]
                + k2m[:, k].astype(np.float64)[:, None]
                * (t_mf[js] + t_rf[js])[None, :]
                + k2r[:, k].astype(np.float64)[:, None] * t_mf[js][None, :]
                + (k3m[:, k] .astype(np.float64)
                   + k3r[:, k].astype(np.float64))[:, None]
            )
            corr += np.maximum(zsq, 0.0)[tri].sum()

        hd = np.concatenate([coef, basis[:, :, :3072]], axis=2)
        in_maps.append({"basis": basis, "head": hd})
    return in_maps, corr


def kernel(pred, target):
    pred = np.asarray(pred, dtype=np.float32)
    target = np.asarray(target, dtype=np.float32)
    in_maps, corr = _make_in_maps(pred, target)
    nc = _get_nc()
    run_bass_kernel_spmd(nc, in_maps, core_ids=list(range(NCORES)))
    res = run_bass_kernel_spmd(nc, in_maps, core_ids=list(range(NCORES)))
    total = -corr
    for r in res.results:
        total += np.asarray(r["out"], dtype=np.float64).sum()
    K = B * (B - 1) // 2
    return np.float32(BETA * total / K)


# revision 29
# speedup vs baseline: 1.0402x; 1.0402x over previous
"""AdaptiveBoundaryRankingLoss on 8 TRN2 NeuronCores — fp8 DoubleRow rank-10.

Math: loss = sum_{i<j} relu(boundary(|dt|) - (p_i-p_j)*sign(dt)) / K,
  dt = t_i - t_j, boundary(a) = BETA*a/(1+GAMMA*a), K = B(B-1)/2.

Host sorts (pred,target) by target ascending (the loss is a sum over
unordered pairs, so relabeling is free). After sorting, for i>j
(strict lower triangle) sign(t_i - t_j) = +1, so with
m(a) = a/(1+GAMMA*a), a = t_i - t_j >= 0, pc = p/BETA:
  loss = BETA/K * sum_{i>j} relu(m(a) + pc_j - pr_i).

m(a) is approximated per row by a minimax quadratic on a in [0, L_row]:
  m(a) ~= beta_r - (s_r*(a - a0_r))^2.
Expanding the square, the pre-relu value is a rank-4 bilinear form
  z_ij = 1*pc_j + k1_i*t_j^2 + k2_i*t_j + k3_i
(k1 = -s^2, k2 = 2sb, k3 = ubt - b^2, b = s(t_i - a0), ubt = beta_r - pr).
TensorE computes z into PSUM f32. To run the PE at fp8 DoubleRow speed
(0.5 cycles/row — the per-buffer TE-fill must hide inside the consumer
window) each factor is split fp8-main + fp8-residual, and the four
logical products expand to 10 first-order products = one K_phys=5,
2-ktile DoubleRow matmul per 512 columns. Second-order (res*res) terms
are dropped (~0.4% of a term, bf16-like accuracy).

Triangular masking: the basis pc rows are baked PER TILE with
pc = -240 (fp8 max is 240) for columns >= 128*(row_block+1), so padded
columns give z < 0 and relu -> 0 exactly. The remaining over-count (the
j >= i half of each block's own 128x128 diagonal square) is computed on
the host from the same fp8 values and subtracted.

PSUM consumption (relu + row-sum in ONE fused op per 2048-col chunk):
  ScalarE: activation(Relu, accum_out)     — 9 of 18 chunks (evens)
  VectorE: tensor_scalar(max 0, accum_out) — 9 of 18 chunks (odds)
Two [128,2048] PSUM buffers (4 banks each) double-buffer TensorE against
the consumers. Per-chunk row-sums land in acc[128,18] f32, DMA'd out;
the host does the final reduce and subtracts the triangle correction.

Work split: 64 row-blocks of 128 rows; core c takes blocks {8k+c},
tile k spans columns [0,(k+1)*1024) -> identical graph on all cores
(SPMD); per-core differences live in input data (basis + coeffs).
The kernel executes the NEFF twice and returns the second (warm) run.
"""

import contextlib

import numpy as np
import ml_dtypes

import concourse.bass as bass
from concourse import mybir
from concourse.bass_utils import run_bass_kernel_spmd

B = 8192
BETA = 0.3
GAMMA = 0.1
NCORES = 8
NT = 8            # tiles per core (one 128-row block each)
P = 128
TOT = 36864       # sum of tile widths (k+1)*1024
CHUNK = 2048      # consumer chunk width (4 PSUM banks)
NCH = TOT // CHUNK  # 18
MMW = 512         # matmul moving max (output columns per matmul)
KR = 5            # K_phys of the DoubleRow matmul (10 effective rows)
MASK = -240.0     # fp8e4 most-negative finite

# tile column offsets in the concatenated basis
OFFS = [0]
for _k in range(NT):
    OFFS.append(OFFS[-1] + (_k + 1) * 1024)  # [0,1024,3072,...,36864]

# every chunk is consumed by BOTH engines in parallel: ScalarE takes
# cols [0, SEW), VectorE [SEW, CHUNK). VE's signal gates TensorE's
# buffer reuse with the least runway, so VE gets the smaller share
SEW = 1152

# basis is stored per tile (ktile stride = tile width must fit a 16-bit
# ISA step field); DMA'd tile by tile in order

_bf16 = ml_dtypes.bfloat16
_f8 = ml_dtypes.float8_e4m3

_NC_CACHE = None


def _tile_of(col):
    for k in range(NT):
        if col < OFFS[k + 1]:
            return k
    raise ValueError(col)


def build_nc():
    nc = bass.Bass(target_bir_lowering=False, debug=False)
    f32 = mybir.dt.float32
    bf16 = mybir.dt.bfloat16
    f8 = mybir.dt.float8e4
    A = mybir.AluOpType

    # head = coef (NT*P) ++ basis tile0 (1024) ++ basis tile1 (2048):
    # one DMA covers everything the first ~3.4us of matmuls need
    HEADW = NT * P + 1024 + 2048
    basis_d = nc.declare_dram_parameter("basis", [KR, 2, TOT], f8, isOutput=False)
    head_d = nc.declare_dram_parameter("head", [KR, 2, HEADW], f8, isOutput=False)
    out_d = nc.declare_dram_parameter("out", [P, 2 * NCH], f32, isOutput=True)

    es = contextlib.ExitStack()
    with es:
        def sb(name, shape, dtype):
            return es.enter_context(nc.sbuf_tensor(name, shape, dtype))

        head = sb("head_s", [KR, 2, HEADW], f8)
        coef = head[:, :, :NT * P]
        basis_t = [
            head[:, :, NT * P:NT * P + 1024],
            head[:, :, NT * P + 1024:],
        ] + [
            sb(f"basis_s{k}", [KR, 2, (k + 1) * 1024], f8)
            for k in range(2, NT)
        ]
        scr_se = sb("scr_se", [P, SEW], bf16)
        scr_ve = sb("scr_ve", [P, CHUNK - SEW], bf16)
        acc = sb("acc", [P, 2 * NCH], f32)
        pa = es.enter_context(nc.psum_tensor("pa", [P, CHUNK], f32))
        pb = es.enter_context(nc.psum_tensor("pb", [P, CHUNK], f32))
        dma_sem = es.enter_context(nc.semaphore("dma_sem"))
        dma_b = es.enter_context(nc.semaphore("dma_b"))
        te_sem = es.enter_context(nc.semaphore("te_sem"))
        se_sem = es.enter_context(nc.semaphore("se_sem"))
        ve_sem = es.enter_context(nc.semaphore("ve_sem"))
        block = es.enter_context(nc.Block())

        pbufs = [pa, pb]

        @block.sync
        def _(sync):
            # head first (coef + tiles 0-1), then odd remaining tiles;
            # even remaining tiles ride the gpsimd queue in parallel
            sync.dma_start(out=head[:, :, :], in_=head_d[:, :, :]).then_inc(
                dma_sem, 16)
            for k in (3, 5, 7):
                lo, hi = OFFS[k], OFFS[k + 1]
                sync.dma_start(
                    out=basis_t[k][:, :, :], in_=basis_d[:, :, lo:hi]
                ).then_inc(dma_sem, 16)
            sync.wait_ge(se_sem, NCH)
            sync.wait_ge(ve_sem, NCH)
            sync.dma_start(out=out_d[:, :], in_=acc[:, :]).then_inc(dma_sem, 16)

        @block.gpsimd
        def _(gpsimd):
            for k in (2, 4, 6):
                lo, hi = OFFS[k], OFFS[k + 1]
                gpsimd.dma_start(
                    out=basis_t[k][:, :, :], in_=basis_d[:, :, lo:hi]
                ).then_inc(dma_b, 16)

        # DMA sem threshold a tile's matmuls must wait for, per queue
        DMA_Q = {0: (dma_sem, 16), 1: (dma_sem, 16),
                 3: (dma_sem, 32), 5: (dma_sem, 48), 7: (dma_sem, 64),
                 2: (dma_b, 16), 4: (dma_b, 32), 6: (dma_b, 48)}

        @block.tensor
        def _(tensor):
            tensor.wait_ge(dma_sem, 16)  # head (coef + tiles 0-1)
            seen_tile = -1
            for c in range(NCH):
                # buffer reuse: wait until chunk c-2's consumers are done
                if c >= 2:
                    tensor.wait_ge(se_sem, c - 1)
                    tensor.wait_ge(ve_sem, c - 1)
                ps = pbufs[c % 2]
                for s in range(CHUNK // MMW):
                    col = c * CHUNK + s * MMW
                    k = _tile_of(col)
                    if k > seen_tile:
                        seen_tile = k
                        sem, thr = DMA_Q[k]
                        tensor.wait_ge(sem, thr)
                    mm = tensor.matmul(
                        ps[:, s * MMW:(s + 1) * MMW],
                        coef[:, :, k * P:(k + 1) * P],
                        basis_t[k][:, :, col - OFFS[k]:col - OFFS[k] + MMW],
                        start=True,
                        stop=True,
                        perf_mode=mybir.MatmulPerfMode.DoubleRow,
                    )
                    mm.then_inc(te_sem, 1)

        @block.scalar
        def _(scalar):
            # dummy 1-elem Relu: pulls ACT_TABLE_LOAD to t=0
            scalar.activation(
                scr_se[:, 0:1], scr_se[:, 0:1],
                mybir.ActivationFunctionType.Relu,
            )
            for c in range(NCH):
                # SE needs only the first SEW cols = first 3 matmuls
                scalar.wait_ge(te_sem, 4 * c + 3)
                scalar.activation(
                    scr_se[:, :], pbufs[c % 2][:, :SEW],
                    mybir.ActivationFunctionType.Relu,
                    accum_out=acc[:, 2 * c:2 * c + 1],
                ).then_inc(se_sem, 1)

        @block.vector
        def _(vector):
            for c in range(NCH):
                vector.wait_ge(te_sem, 4 * (c + 1))
                vector.tensor_scalar(
                    out=scr_ve[:, :], in0=pbufs[c % 2][:, SEW:],
                    scalar1=0.0, scalar2=None, op0=A.max, op1=A.add,
                    accum_out=acc[:, 2 * c + 1:2 * c + 2],
                ).then_inc(ve_sem, 1)

    return nc


def _get_nc():
    global _NC_CACHE
    if _NC_CACHE is None:
        _NC_CACHE = build_nc()
    return _NC_CACHE


def _quad_fit_rows(L, n=48):
    """Vectorized per-row quadratic fit of m(a)=a/(1+G*a) on [0, L_r]
    via Chebyshev interpolation (degree 2). Returns coeff arrays
    (c0, c1, c2) of p(a) = c0 + c1*a + c2*a^2."""
    L = np.maximum(np.asarray(L, np.float64), 1e-3)
    n_ = n
    xk = np.cos((2 * np.arange(n_) + 1) * np.pi / (2 * n_))
    a = (xk[None, :] + 1.0) * 0.5 * L[:, None]          # [rows, n]
    f = a / (1.0 + GAMMA * a)
    b0 = f @ (np.ones_like(xk) / n_)
    b1 = f @ (xk * 2.0 / n_)
    b2 = f @ ((2 * xk * xk - 1.0) * 2.0 / n_)
    # p(x) = (b0 - b2) + b1*x + 2*b2*x^2,  x = 2a/L - 1
    A0 = b0 - b2
    A1 = b1
    A2 = 2 * b2
    c0 = A0 - A1 + A2
    c1 = (A1 - 2 * A2) * 2.0 / L
    c2 = A2 * 4.0 / (L * L)
    return c0, c1, c2


def _split8(x):
    """fp8 main + fp8 residual decomposition (returns fp8 arrays)."""
    m = np.asarray(x, np.float64).astype(_f8)
    r = (np.asarray(x, np.float64) - m.astype(np.float64)).astype(_f8)
    return m, r


def _make_in_maps(pred, target):
    """Returns (in_maps, corr) where corr is the host-side sum of
    relu(z) over the j>=i part of every block's diagonal 128x128
    square (exactly what the device over-counts)."""
    order = np.argsort(target, kind="stable")
    t = target[order].astype(np.float64)
    p = pred[order].astype(np.float64)
    tmin = t[0]

    # shared basis data: fp8 main + residual of pc, t^2, t
    pc_m, pc_r = _split8(p / BETA)
    t2_m, t2_r = _split8(t * t)
    t_m, t_r = _split8(t)
    ones8 = np.ones(B, dtype=_f8)
    # f64 views for the host correction
    pc_mf, pc_rf = pc_m.astype(np.float64), pc_r.astype(np.float64)
    t2_mf, t2_rf = t2_m.astype(np.float64), t2_r.astype(np.float64)
    t_mf, t_rf = t_m.astype(np.float64), t_r.astype(np.float64)

    in_maps = []
    corr = 0.0
    jj = np.arange(P)
    tri = jj[None, :] >= jj[:, None]     # within-block j >= i (incl diag)
    for c in range(NCORES):
        rows = (8 * np.arange(NT)[None, :] + c) * P + np.arange(P)[:, None]
        tr = t[rows]                      # [128, 8]
        pr = p[rows] / BETA
        c0, c1, c2 = _quad_fit_rows((tr - tmin).ravel())
        c0 = c0.reshape(P, NT)
        c1 = c1.reshape(P, NT)
        c2 = np.minimum(c2.reshape(P, NT), -1e-8)
        s = np.sqrt(-c2)
        a0 = -c1 / (2 * c2)
        beta_r = c0 - c2 * a0 * a0
        b = s * (tr - a0)
        ubt = beta_r - pr
        k1m, k1r = _split8(-(s * s))      # [128, 8] each
        k2m, k2r = _split8(2 * s * b)
        k3m, k3r = _split8(ubt - b * b)

        # coef rows (K_phys=5, 2 ktiles): pairing per module docstring
        coef = np.zeros((KR, 2, NT * P), dtype=_f8)
        for k in range(NT):
            sl = slice(k * P, (k + 1) * P)
            coef[0, 0, sl] = _f8(1.0)
            coef[0, 1, sl] = _f8(1.0)
            coef[1, 0, sl] = k1m[:, k]
            coef[1, 1, sl] = k1m[:, k]
            coef[2, 0, sl] = k1r[:, k]
            coef[2, 1, sl] = k2m[:, k]
            coef[3, 0, sl] = k2m[:, k]
            coef[3, 1, sl] = k2r[:, k]
            coef[4, 0, sl] = k3m[:, k]
            coef[4, 1, sl] = k3r[:, k]

        basis = np.empty((KR, 2, TOT), dtype=_f8)
        for k in range(NT):
            lo, hi = OFFS[k], OFFS[k + 1]
            w = hi - lo
            r = 8 * k + c
            jmax = P * (r + 1)            # valid cols are j < jmax
            pm = pc_m[:w].copy()
            prs = pc_r[:w].copy()
            if jmax < w:
                pm[jmax:] = _f8(MASK)
                prs[jmax:] = _f8(MASK)
            basis[0, 0, lo:hi] = pm
            basis[0, 1, lo:hi] = prs
            basis[1, 0, lo:hi] = t2_m[:w]
            basis[1, 1, lo:hi] = t2_r[:w]
            basis[2, 0, lo:hi] = t2_m[:w]
            basis[2, 1, lo:hi] = t_r[:w]
            basis[3, 0, lo:hi] = t_m[:w]
            basis[3, 1, lo:hi] = t_m[:w]
            basis[4, 0, lo:hi] = ones8[:w]
            basis[4, 1, lo:hi] = ones8[:w]

            # host correction for this block's diagonal square
            j0 = P * r
            js = slice(j0, j0 + P)
            zsq = (
                pc_mf[js][None, :] + pc_rf[js][None, :]
                + k1m[:, k].astype(np.float64)[:, None]
                * (t2_mf[js] + t2_rf[js])[None, :]
                + k1r[:, k].astype(np.float64)[:, None] * t2_mf[js][None, :]
                + k2m[:, k].astype(np.float64)[:, None]
                * (t_mf[js] + t_rf[js])[None, :]
                + k2r[:, k].astype(np.float64)[:, None] * t_mf[js][None, :]
                + (k3m[:, k] .astype(np.float64)
                   + k3r[:, k].astype(np.float64))[:, None]
            )
            corr += np.maximum(zsq, 0.0)[tri].sum()

        hd = np.concatenate([coef, basis[:, :, :3072]], axis=2)
        in_maps.append({"basis": basis, "head": hd})
    return in_maps, corr


def kernel(pred, target):
    pred = np.asarray(pred, dtype=np.float32)
    target = np.asarray(target, dtype=np.float32)
    in_maps, corr = _make_in_maps(pred, target)
    nc = _get_nc()
    run_bass_kernel_spmd(nc, in_maps, core_ids=list(range(NCORES)))
    res = run_bass_kernel_spmd(nc, in_maps, core_ids=list(range(NCORES)))
    total = -corr
    for r in res.results:
        total += np.asarray(r["out"], dtype=np.float64).sum()
    K = B * (B - 1) // 2
    return np.float32(BETA * total / K)
